# revision 15
# baseline (speedup 1.0000x reference)
"""Trainium2 Bass kernel for GNN message passing (IntraConv + BatchNorm).

Computation (reference):
    msg   = feat[src] * edge_weight                    [E, D]
    neigh = segment_sum(msg, dst, N)                   [N, D]
    deg   = segment_sum(edge_weight, dst, N)           [N, 1]
    h     = relu(feat @ Ws.T + b_self + (neigh/(deg+eps)) @ Wn.T + bias)
    out   = batchnorm(h; gamma, beta)  (training-mode batch stats)

Distribution over 8 NeuronCores: edges are sorted by dst and sharded by
dst-range so each core owns N/8 contiguous nodes and every edge pointing at
them.  Local segment sums are then exact — the only collective is an
AllReduce of the [128, 2] BatchNorm statistics.

Per-core pipeline (feature-major):
  - dma_gather of fp32 feature rows (512B) per 128-dst tile.  dma_gather
    indices are int16, so the node table is split at 32768 and each tile
    does a lo-gather and a hi-gather into one [128, K, 128] buffer.
  - one-hot S[e, d] = (dstl[e] == d) built in bf16 with a single is_equal;
    PE matmuls accumulate S.T @ (w * G) into PSUM [128 nodes, 128] (neigh)
    and S.T @ w into PSUM [128, 1] (degree).
  - normalize by 1/(deg+eps) (per-partition scalar), transpose to
    feature-major via PE, assemble h_neigh.T slab.
  - linears with stationary W.T (bf16); bias+relu and BN partial stats on
    the ACT engine (activation Relu/Square with accum_out); tiny AllReduce;
    scale/shift; output written feature-major [128, N/8] and transposed on
    the host during unshard.
"""

import numpy as np
import ml_dtypes
from contextlib import ExitStack

import concourse.bass as bass
import concourse.tile as tile
from concourse import bacc, mybir
from concourse.bass_utils import run_bass_kernel_spmd
from concourse.masks import make_identity

N_CORES = 8
P = 128
HALF = 32768        # int16 index limit for dma_gather
LIN_CHUNK = 512
EPS_DEG = 1e-8
EPS_BN = 1e-5

F32 = mybir.dt.float32
BF16 = mybir.dt.bfloat16
I16 = mybir.dt.int16
OP = mybir.AluOpType
ACT = mybir.ActivationFunctionType


def _bcast_inner(ap, n):
    """[.., M] -> [.., M, n] with stride-0 inner broadcast dim."""
    return bass.AP(tensor=ap.tensor, offset=ap.offset, ap=list(ap.ap) + [[0, n]])


def _bcast_mid(ap2d, k):
    """[Pp, M] -> [Pp, k(bcast), M]."""
    a = list(ap2d.ap)
    return bass.AP(tensor=ap2d.tensor, offset=ap2d.offset, ap=[a[0], [0, k], a[1]])


def _host_plan(feat, src, dst, edge_weight):
    N, D = feat.shape
    E = src.shape[0]
    assert D == P and N % N_CORES == 0
    npc = N // N_CORES                      # nodes per core
    T = (npc + P - 1) // P                  # dst tiles per core
    nw = T * P                              # padded node-slab width
    n_hi = N - HALF if N > HALF else 0

    src64 = src.astype(np.int64)
    dst64 = dst.astype(np.int64)
    ws_all = edge_weight.reshape(-1).astype(np.float32)

    half = (src64 >= HALF).astype(np.int64)
    ct = (dst64 // npc) * T + (dst64 % npc) // P      # (core, tile) group id
    order = np.lexsort((half, ct))
    ss = src64[order]
    ws = ws_all[order]
    hh = half[order]
    cts = ct[order]
    dstl = ((dst64[order] % npc) % P).astype(np.float32)

    grp = cts * 2 + hh                                 # (core, tile, half)
    counts = np.bincount(grp, minlength=N_CORES * T * 2)
    cnt_lo = counts[0::2]
    cnt_hi = counts[1::2]
    K_LO = max(1, int(np.ceil(cnt_lo.max() / P)))
    K_HI = max(1, int(np.ceil(cnt_hi.max() / P))) if n_hi > 0 else 0
    K = K_LO + K_HI
    ET = K * P

    starts = np.zeros(N_CORES * T * 2 + 1, np.int64)
    np.cumsum(counts, out=starts[1:])
    pos = np.arange(E, dtype=np.int64) - starts[grp]
    q = pos + hh * (K_LO * P)                          # stream position in tile
    flat = cts * ET + q

    # pad slots: idx=0 (gathered but harmless); their S_w row stays all-zero so
    # pad rows never contribute.  idx=-1 skip-padding would need num_idxs_reg
    # to be the per-core valid count, which a shared SPMD immediate can't hold.
    idx_stream = np.zeros(N_CORES * T * ET, np.int32)
    idx_stream[flat] = ss - hh * HALF

    # host-built weighted one-hot: S_w[core, p, t*K + c, dstl] = w for the edge
    # at stream position q = c*128 + p of (core, tile).  Pure placement of w
    # values (no arithmetic); replaces the per-tile DVE is_equal/mult build.
    core_of = cts // T
    t_of = cts % T
    c_of = q // P
    p_of = q % P
    sw_sb = np.zeros((N_CORES, T, P, K, P), ml_dtypes.bfloat16)
    sw_sb[core_of, t_of, p_of, c_of, dstl.astype(np.int64)] = ws.astype(
        ml_dtypes.bfloat16
    )
    # tile-major contiguous: each tile's [128, K*128] block is one flat DRAM
    # run so its load coalesces into 16 large descriptors, not a 128-way spray
    sw_sb = np.ascontiguousarray(sw_sb.reshape(N_CORES, T * P, K * P))

    # gather indices: [16-wrap, replicate x8] per (tile, half)
    def wrap(a):  # [N_CORES, T, n] -> [N_CORES, 128, T, n//16]
        c0, t0, n = a.shape
        a = a.reshape(c0, t0, n // 16, 16).transpose(0, 3, 1, 2)
        return np.tile(a, (1, 8, 1, 1))

    ist = idx_stream.reshape(N_CORES, T, ET)
    parts = [wrap(ist[:, :, : K_LO * P])]
    if K_HI > 0:
        parts.append(wrap(ist[:, :, K_LO * P:]))
    idx_sb = np.concatenate(parts, axis=3)             # [N_CORES, 128, T, K*8]
    idx_sb = np.ascontiguousarray(
        idx_sb.reshape(N_CORES, P, T * K * 8)
    ).astype(np.int16)

    # bf16 tables with 256-elem rows: [feat(128), 1.0, zeros(127)].
    # 512B/row keeps the gather at full descriptor rate; the 1.0 column makes
    # the degree fall out of the neighbor matmul (rhs column 128).
    def table(rows):
        t = np.zeros((max(rows.shape[0], 1), 256), ml_dtypes.bfloat16)
        t[: rows.shape[0], :P] = rows.astype(ml_dtypes.bfloat16)
        t[:, P] = 1.0
        return t

    feat_lo = table(feat[:HALF])
    feat_hi = table(feat[HALF:]) if n_hi > 0 else np.zeros((1, 256), ml_dtypes.bfloat16)

    # per-core self-feature slab, bf16, zero padded to nw rows
    feat_self = np.zeros((N_CORES, nw, P), ml_dtypes.bfloat16)
    fb = feat.reshape(N_CORES, npc, P)
    for c in range(N_CORES):
        feat_self[c, :npc] = fb[c]

    return dict(
        N=N, E=E, npc=npc, T=T, K_LO=K_LO, K_HI=K_HI, nw=nw,
        n_lo=min(N, HALF), n_hi=max(n_hi, 1),
        idx_sb=idx_sb, sw_sb=sw_sb,
        feat_lo=feat_lo, feat_hi=feat_hi,
        feat_self=feat_self,
    )


def _build_program(N, T, K_LO, K_HI, npc, nw, n_lo, n_hi, n_cores=N_CORES,
                   reps=1, ablate=frozenset()):
    K = K_LO + K_HI
    K8 = K * 8
    nc = bacc.Bacc(
        "TRN2",
        target_bir_lowering=False,
        debug=False,
        enable_asserts=False,
        num_devices=n_cores,
        num_swdge_queues=4,
    )

    flo_d = nc.dram_tensor("feat_lo", [n_lo, 256], BF16, kind="ExternalInput")
    fhi_d = nc.dram_tensor("feat_hi", [n_hi, 256], BF16, kind="ExternalInput")
    idx_d = nc.dram_tensor("idx_sb", [P, T * K8], I16, kind="ExternalInput")
    sw_d = nc.dram_tensor("sw_sb", [T * P, K * P], BF16, kind="ExternalInput")
    fself_d = nc.dram_tensor("feat_self", [nw, P], BF16, kind="ExternalInput")
    wn_d = nc.dram_tensor("wn_t", [P, P], BF16, kind="ExternalInput")
    ws_d = nc.dram_tensor("ws_t", [P, P], BF16, kind="ExternalInput")
    bias_d = nc.dram_tensor("bias_sum", [P, 1], F32, kind="ExternalInput")
    gamma_d = nc.dram_tensor("gamma_c", [P, 1], F32, kind="ExternalInput")
    beta_d = nc.dram_tensor("beta_c", [P, 1], F32, kind="ExternalInput")

    nchunks_out = (npc + LIN_CHUNK - 1) // LIN_CHUNK
    out_d = nc.dram_tensor("outT", [nchunks_out * P, LIN_CHUNK], F32,
                           kind="ExternalOutput")

    cc_in = nc.dram_tensor("cc_in", [P, 2], F32)
    cc_out = nc.dram_tensor("cc_out", [P, 2], F32, addr_space="Shared")

    with tile.TileContext(nc) as tc, ExitStack() as ctx:
        const = ctx.enter_context(tc.tile_pool(name="const", bufs=1))
        slabs = ctx.enter_context(tc.tile_pool(name="slabs", bufs=1))
        gpool = ctx.enter_context(tc.tile_pool(name="gpool", bufs=6))
        spool = ctx.enter_context(tc.tile_pool(name="spool", bufs=4))
        hnpool = ctx.enter_context(tc.tile_pool(name="hnpool", bufs=3))
        small = ctx.enter_context(tc.tile_pool(name="small", bufs=6))
        stage = ctx.enter_context(tc.tile_pool(name="stage", bufs=3))
        ps_acc = ctx.enter_context(tc.tile_pool(name="ps_acc", bufs=2, space="PSUM"))
        ps_tr = ctx.enter_context(tc.tile_pool(name="ps_tr", bufs=2, space="PSUM"))
        ps_lin = ctx.enter_context(tc.tile_pool(name="ps_lin", bufs=2, space="PSUM"))

        # ---- constants ----
        idx_t = const.tile([P, T * K8], I16)
        IQ = (T + 3) // 4 * K8
        for j in range(4):
            j0 = j * IQ
            j1 = min(T * K8, j0 + IQ)
            if j1 > j0:
                nc.sync.dma_start(idx_t[:, j0:j1], idx_d[:, j0:j1])
        wn_t = const.tile([P, P], BF16)
        nc.sync.dma_start(wn_t[:], wn_d[:, :])
        ws_t = const.tile([P, P], BF16)
        nc.sync.dma_start(ws_t[:], ws_d[:, :])
        bias_t = const.tile([P, 1], F32)
        nc.sync.dma_start(bias_t[:], bias_d[:, :])
        gamma_t = const.tile([P, 1], F32)
        nc.sync.dma_start(gamma_t[:], gamma_d[:, :])
        beta_t = const.tile([P, 1], F32)
        nc.sync.dma_start(beta_t[:], beta_d[:, :])
        ident = const.tile([P, P], BF16)
        make_identity(nc, ident[:])

        featT = slabs.tile([P, nw], BF16)
        nc.sync.dma_start_transpose(featT[:], fself_d[:, :])
        rst = slabs.tile([P, nw], F32)
        hnT = slabs.tile([P, nw], BF16)

        # ablation shrink factors (bench-only; full kernel uses none)
        AB_G = "gather" in ablate
        AB_MM = "mm" in ablate
        AB_EPI = "epi" in ablate

        gq = [0]  # round-robin SWDGE queue so desc-gen pipelines across Q7 core pairs

        nchunks = (nw + LIN_CHUNK - 1) // LIN_CHUNK

        def lin_chunk(j, sum_parts, sq_parts):
            """fc_self + fc_neigh + bias + relu + BN partial stats for column
            chunk j.  Interleaved into the tile loop so the epilogue overlaps
            message passing instead of trailing it."""
            c0 = j * LIN_CHUNK
            cw = min(LIN_CHUNK, nw - c0)
            vw = min(max(npc - c0, 0), cw)          # valid (non-pad) columns
            pl = ps_lin.tile([P, LIN_CHUNK], F32, space="PSUM")
            nc.tensor.matmul(
                out=pl[:, 0:cw], lhsT=ws_t[:], rhs=featT[:, c0:c0 + cw],
                start=True, stop=False,
            )
            nc.tensor.matmul(
                out=pl[:, 0:cw], lhsT=wn_t[:], rhs=hnT[:, c0:c0 + cw],
                start=False, stop=True,
            )
            # rst = relu(pl + bias); partial sums on ACT
            nc.scalar.activation(
                out=rst[:, c0:c0 + cw], in_=pl[:, 0:cw], func=ACT.Relu,
                bias=bias_t[:],
            )
            if vw > 0:
                nc.vector.tensor_reduce(
                    out=sum_parts[:, j:j + 1], in_=rst[:, c0:c0 + vw],
                    axis=mybir.AxisListType.X, op=OP.add,
                )
                junk = stage.tile([P, LIN_CHUNK], F32, tag="junk")
                nc.scalar.activation(
                    out=junk[:, 0:vw], in_=rst[:, c0:c0 + vw], func=ACT.Square,
                    accum_out=sq_parts[:, j:j + 1],
                )
            else:
                nc.vector.memset(sum_parts[:, j:j + 1], 0.0)
                nc.vector.memset(sq_parts[:, j:j + 1], 0.0)

        for _rep in range(reps):
            sum_parts = small.tile([P, nchunks], F32, tag="sump")
            sq_parts = small.tile([P, nchunks], F32, tag="sqp")
            emitted = [0]
            # ---- message passing per dst tile ----
            GMAX = 8  # dma_gather is limited to 1024 indices per instruction
            for t in range(T):
                g = gpool.tile([P, K, 256], BF16)
                for tab, k0, kn in ((flo_d, 0, K_LO), (fhi_d, K_LO, K_HI)):
                    for cb in range(0, kn, GMAX):
                        cn = 1 if AB_G else min(GMAX, kn - cb)
                        nc.gpsimd.dma_gather(
                            out_ap=g[:, k0 + cb:k0 + cb + cn, :],
                            in_ap=tab.ap(),
                            idxs_ap=idx_t[:, t * K8 + (k0 + cb) * 8:
                                          t * K8 + (k0 + cb + cn) * 8],
                            num_idxs=cn * P,
                            num_idxs_reg=cn * P,
                            elem_size=256,
                            queue_num=gq[0] % 4,
                        )
                        gq[0] += 1
                # host-built weighted one-hot for this tile
                s = spool.tile([P, K * P], BF16)
                nc.sync.dma_start(s[:], sw_d[t * P:(t + 1) * P, :])
                # accumulate [dst, feat | deg]: rhs column 128 is the 1.0 pad
                ps = ps_acc.tile([P, P + 1], F32, space="PSUM")
                K_MM = 1 if AB_MM else K
                for c in range(K_MM):
                    nc.tensor.matmul(
                        out=ps[:],
                        lhsT=s[:, c * P:(c + 1) * P],
                        rhs=g[:, c, 0:P + 1],
                        start=(c == 0),
                        stop=(c == K_MM - 1),
                    )
                dinv = small.tile([P, 1], F32, tag="dinv")
                nc.vector.tensor_scalar(
                    out=dinv[:], in0=ps[:, P:P + 1], scalar1=EPS_DEG, scalar2=None,
                    op0=OP.add,
                )
                nc.vector.reciprocal(dinv[:], dinv[:])
                hn = hnpool.tile([P, P], BF16)
                nc.scalar.activation(
                    out=hn[:], in_=ps[:, 0:P], func=ACT.Copy, scale=dinv[:],
                )
                pst = ps_tr.tile([P, P], BF16, space="PSUM")
                nc.tensor.transpose(out=pst[:], in_=hn[:], identity=ident[:])
                nc.scalar.copy(hnT[:, t * P:(t + 1) * P], pst[:])

                # epilogue chunk j is ready once its 4 source tiles are done
                while (emitted[0] + 1) * (LIN_CHUNK // P) <= t + 1 \
                        and emitted[0] < nchunks:
                    lin_chunk(emitted[0], sum_parts, sq_parts)
                    emitted[0] += 1

            while emitted[0] < nchunks:
                lin_chunk(emitted[0], sum_parts, sq_parts)
                emitted[0] += 1

            stats = small.tile([P, 2], F32, tag="stats")
            nc.vector.tensor_reduce(
                out=stats[:, 0:1], in_=sum_parts[:, 0:nchunks],
                axis=mybir.AxisListType.X, op=OP.add
            )
            nc.vector.tensor_reduce(
                out=stats[:, 1:2], in_=sq_parts[:, 0:nchunks],
                axis=mybir.AxisListType.X, op=OP.add
            )
            nc.sync.dma_start(cc_in[:, :], stats[:])
            nc.gpsimd.collective_compute(
                "AllReduce",
                OP.add,
                replica_groups=[list(range(n_cores))],
                ins=[cc_in.ap().opt()],
                outs=[cc_out.ap().opt()],
            )
            gstats = small.tile([P, 2], F32, tag="gstats")
            nc.sync.dma_start(gstats[:], cc_out[:, :])

            # ---- BN scale/shift ----
            inv_n = 1.0 / N
            mu = small.tile([P, 1], F32, tag="mu")
            nc.vector.tensor_scalar(
                out=mu[:], in0=gstats[:, 0:1], scalar1=inv_n, scalar2=None, op0=OP.mult
            )
            var = small.tile([P, 1], F32, tag="var")
            nc.vector.tensor_scalar(
                out=var[:], in0=gstats[:, 1:2], scalar1=inv_n, scalar2=None, op0=OP.mult
            )
            mu2 = small.tile([P, 1], F32, tag="mu2")
            nc.vector.tensor_tensor(out=mu2[:], in0=mu[:], in1=mu[:], op=OP.mult)
            nc.vector.tensor_tensor(out=var[:], in0=var[:], in1=mu2[:], op=OP.subtract)
            eps_t = small.tile([P, 1], F32, tag="eps")
            nc.vector.memset(eps_t[:], EPS_BN)
            std = small.tile([P, 1], F32, tag="std")
            nc.scalar.activation(out=std[:], in_=var[:], func=ACT.Sqrt, bias=eps_t[:])
            rstd = small.tile([P, 1], F32, tag="rstd")
            nc.vector.reciprocal(rstd[:], std[:])
            scale = small.tile([P, 1], F32, tag="scale")
            nc.vector.tensor_tensor(out=scale[:], in0=gamma_t[:], in1=rstd[:], op=OP.mult)
            shift = small.tile([P, 1], F32, tag="shift")
            nc.vector.tensor_tensor(out=shift[:], in0=mu[:], in1=scale[:], op=OP.mult)
            nc.vector.tensor_tensor(out=shift[:], in0=beta_t[:], in1=shift[:], op=OP.subtract)

            # ---- apply + write out (contiguous chunk layout) ----
            for j in range(1 if AB_EPI else (npc + LIN_CHUNK - 1) // LIN_CHUNK):
                c0 = j * LIN_CHUNK
                cw = min(LIN_CHUNK, npc - c0)
                ot = stage.tile([P, LIN_CHUNK], F32, tag="ostage")
                nc.vector.tensor_scalar(
                    out=ot[:, 0:cw], in0=rst[:, c0:c0 + cw],
                    scalar1=scale[:], scalar2=shift[:], op0=OP.mult, op1=OP.add,
                )
                nc.sync.dma_start(out_d[j * P:(j + 1) * P, 0:cw], ot[:, 0:cw])

    nc.compile()
    return nc


_cache = {}


def _get_program(key_params):
    key = tuple(sorted(key_params.items()))
    if key not in _cache:
        _cache[key] = _build_program(**key_params)
    return _cache[key]


def _in_maps(plan, W_neigh, W_self, b_self, bias, gamma, beta):
    wn_t = np.ascontiguousarray(W_neigh.T).astype(ml_dtypes.bfloat16)
    ws_t = np.ascontiguousarray(W_self.T).astype(ml_dtypes.bfloat16)
    bias_sum = (np.asarray(b_self) + np.asarray(bias)).astype(np.float32).reshape(P, 1)
    maps = []
    for c in range(N_CORES):
        maps.append({
            "feat_lo": plan["feat_lo"],
            "feat_hi": plan["feat_hi"],
            "idx_sb": plan["idx_sb"][c],
            "sw_sb": plan["sw_sb"][c],
            "feat_self": plan["feat_self"][c],
            "wn_t": wn_t,
            "ws_t": ws_t,
            "bias_sum": bias_sum,
            "gamma_c": np.asarray(gamma, np.float32).reshape(P, 1),
            "beta_c": np.asarray(beta, np.float32).reshape(P, 1),
        })
    return maps


def kernel(feat, src, dst, edge_weight, W_neigh, W_self, b_self, bias, gamma, beta):
    N, D = feat.shape
    plan = _host_plan(
        np.asarray(feat), np.asarray(src), np.asarray(dst), np.asarray(edge_weight)
    )
    npc = plan["npc"]

    nc = _get_program(dict(
        N=N, T=plan["T"], K_LO=plan["K_LO"], K_HI=plan["K_HI"],
        npc=npc, nw=plan["nw"], n_lo=plan["n_lo"], n_hi=plan["n_hi"],
    ))

    maps = _in_maps(plan, W_neigh, W_self, b_self, bias, gamma, beta)
    res = run_bass_kernel_spmd(nc, maps, core_ids=list(range(N_CORES)))
    out = np.empty((N, P), np.float32)
    ncho = (npc + LIN_CHUNK - 1) // LIN_CHUNK
    for c in range(N_CORES):
        chunks = res.results[c]["outT"].reshape(ncho, P, LIN_CHUNK)
        outT = np.concatenate(
            [chunks[j][:, :min(LIN_CHUNK, npc - j * LIN_CHUNK)]
             for j in range(ncho)], axis=1,
        )
        out[c * npc:(c + 1) * npc] = outT.T
    return out



# revision 18
# speedup vs baseline: 1.7263x; 1.7263x over previous
"""Trainium2 Bass kernel for GNN message passing (IntraConv + BatchNorm).

Computation (reference):
    msg   = feat[src] * edge_weight                    [E, D]
    neigh = segment_sum(msg, dst, N)                   [N, D]
    deg   = segment_sum(edge_weight, dst, N)           [N, 1]
    h     = relu(feat @ Ws.T + b_self + (neigh/(deg+eps)) @ Wn.T + bias)
    out   = batchnorm(h; gamma, beta)  (training-mode batch stats)

Distribution over 8 NeuronCores: edges are sorted by dst and sharded by
dst-range so each core owns N/8 contiguous nodes and every edge pointing at
them.  Local segment sums are then exact — the only collective is an
AllReduce of the [128, 2] BatchNorm statistics.

Per-core pipeline (feature-major):
  - dma_gather of fp32 feature rows (512B) per 128-dst tile.  dma_gather
    indices are int16, so the node table is split at 32768 and each tile
    does a lo-gather and a hi-gather into one [128, K, 128] buffer.
  - one-hot S[e, d] = (dstl[e] == d) built in bf16 with a single is_equal;
    PE matmuls accumulate S.T @ (w * G) into PSUM [128 nodes, 128] (neigh)
    and S.T @ w into PSUM [128, 1] (degree).
  - normalize by 1/(deg+eps) (per-partition scalar), transpose to
    feature-major via PE, assemble h_neigh.T slab.
  - linears with stationary W.T (bf16); bias+relu and BN partial stats on
    the ACT engine (activation Relu/Square with accum_out); tiny AllReduce;
    scale/shift; output written feature-major [128, N/8] and transposed on
    the host during unshard.
"""

import numpy as np
import ml_dtypes
from contextlib import ExitStack

import concourse.bass as bass
import concourse.tile as tile
from concourse import bacc, mybir
from concourse.bass_utils import run_bass_kernel_spmd
from concourse.masks import make_identity

N_CORES = 8
P = 128
HALF = 32768        # int16 index limit for dma_gather
LIN_CHUNK = 512
EPS_DEG = 1e-8
EPS_BN = 1e-5

F32 = mybir.dt.float32
BF16 = mybir.dt.bfloat16
I16 = mybir.dt.int16
OP = mybir.AluOpType
ACT = mybir.ActivationFunctionType


def _bcast_inner(ap, n):
    """[.., M] -> [.., M, n] with stride-0 inner broadcast dim."""
    return bass.AP(tensor=ap.tensor, offset=ap.offset, ap=list(ap.ap) + [[0, n]])


def _bcast_mid(ap2d, k):
    """[Pp, M] -> [Pp, k(bcast), M]."""
    a = list(ap2d.ap)
    return bass.AP(tensor=ap2d.tensor, offset=ap2d.offset, ap=[a[0], [0, k], a[1]])


def _host_plan(feat, src, dst, edge_weight):
    N, D = feat.shape
    E = src.shape[0]
    assert D == P and N % N_CORES == 0
    npc = N // N_CORES                      # nodes per core
    T = (npc + P - 1) // P                  # dst tiles per core
    nw = T * P                              # padded node-slab width
    n_hi = N - HALF if N > HALF else 0

    src64 = src.astype(np.int64)
    dst64 = dst.astype(np.int64)
    ws_all = edge_weight.reshape(-1).astype(np.float32)

    half = (src64 >= HALF).astype(np.int64)
    ct = (dst64 // npc) * T + (dst64 % npc) // P      # (core, tile) group id
    order = np.lexsort((half, ct))
    ss = src64[order]
    ws = ws_all[order]
    hh = half[order]
    cts = ct[order]
    dstl = ((dst64[order] % npc) % P).astype(np.float32)

    grp = cts * 2 + hh                                 # (core, tile, half)
    counts = np.bincount(grp, minlength=N_CORES * T * 2)
    cnt_lo = counts[0::2]
    cnt_hi = counts[1::2]
    K_LO = max(1, int(np.ceil(cnt_lo.max() / P)))
    K_HI = max(1, int(np.ceil(cnt_hi.max() / P))) if n_hi > 0 else 0
    K = K_LO + K_HI
    ET = K * P

    starts = np.zeros(N_CORES * T * 2 + 1, np.int64)
    np.cumsum(counts, out=starts[1:])
    pos = np.arange(E, dtype=np.int64) - starts[grp]
    q = pos + hh * (K_LO * P)                          # stream position in tile
    flat = cts * ET + q

    # pad slots: idx=-1.  Trailing negatives are skipped by the Q7 desc-gen;
    # num_idxs_reg is reg_loaded per core with the exact valid count so the NX
    # ring bookkeeping matches the Q7's trailing-negative scan.
    idx_stream = np.full(N_CORES * T * ET, -1, np.int32)
    idx_stream[flat] = ss - hh * HALF

    # per-(core, tile, gather-window) valid counts, in gather emission order
    GMAX = 8
    wins = []
    for k0, kn in ((0, K_LO), (K_LO, K_HI)):
        for cb in range(0, kn, GMAX):
            wins.append((k0 + cb, min(GMAX, kn - cb)))
    cl = cnt_lo.reshape(N_CORES, T)
    ch = cnt_hi.reshape(N_CORES, T)
    cnts = np.zeros((N_CORES, T, len(wins)), np.int32)
    for wi, (cb, cn) in enumerate(wins):
        if cb < K_LO:
            base, seg0 = cl, cb
        else:
            base, seg0 = ch, cb - K_LO
        cnts[:, :, wi] = np.clip(base - seg0 * P, 0, cn * P)
    cnts = cnts.reshape(N_CORES, 1, T * len(wins))

    # host-built weighted one-hot: S_w[core, p, t*K + c, dstl] = w for the edge
    # at stream position q = c*128 + p of (core, tile).  Pure placement of w
    # values (no arithmetic); replaces the per-tile DVE is_equal/mult build.
    core_of = cts // T
    t_of = cts % T
    c_of = q // P
    p_of = q % P
    sw_sb = np.zeros((N_CORES, T, P, K, P), ml_dtypes.bfloat16)
    sw_sb[core_of, t_of, p_of, c_of, dstl.astype(np.int64)] = ws.astype(
        ml_dtypes.bfloat16
    )
    # tile-major contiguous: each tile's [128, K*128] block is one flat DRAM
    # run so its load coalesces into 16 large descriptors, not a 128-way spray
    sw_sb = np.ascontiguousarray(sw_sb.reshape(N_CORES, T * P, K * P))

    # gather indices: [16-wrap, replicate x8] per (tile, half)
    def wrap(a):  # [N_CORES, T, n] -> [N_CORES, 128, T, n//16]
        c0, t0, n = a.shape
        a = a.reshape(c0, t0, n // 16, 16).transpose(0, 3, 1, 2)
        return np.tile(a, (1, 8, 1, 1))

    ist = idx_stream.reshape(N_CORES, T, ET)
    parts = [wrap(ist[:, :, : K_LO * P])]
    if K_HI > 0:
        parts.append(wrap(ist[:, :, K_LO * P:]))
    idx_sb = np.concatenate(parts, axis=3)             # [N_CORES, 128, T, K*8]
    idx_sb = np.ascontiguousarray(
        idx_sb.reshape(N_CORES, P, T * K * 8)
    ).astype(np.int16)

    # bf16 tables with 256-elem rows: [feat(128), 1.0, zeros(127)].
    # 512B/row keeps the gather at full descriptor rate; the 1.0 column makes
    # the degree fall out of the neighbor matmul (rhs column 128).
    def table(rows):
        t = np.zeros((max(rows.shape[0], 1), 256), ml_dtypes.bfloat16)
        t[: rows.shape[0], :P] = rows.astype(ml_dtypes.bfloat16)
        t[:, P] = 1.0
        return t

    feat_lo = table(feat[:HALF])
    feat_hi = table(feat[HALF:]) if n_hi > 0 else np.zeros((1, 256), ml_dtypes.bfloat16)

    # per-core self-feature slab, bf16, pre-transposed to [P, nw] so the load
    # is a plain contiguous DMA instead of an xbar transpose
    feat_self = np.zeros((N_CORES, P, nw), ml_dtypes.bfloat16)
    fb = feat.reshape(N_CORES, npc, P)
    for c in range(N_CORES):
        feat_self[c, :, :npc] = fb[c].T

    return dict(
        N=N, E=E, npc=npc, T=T, K_LO=K_LO, K_HI=K_HI, nw=nw,
        n_lo=min(N, HALF), n_hi=max(n_hi, 1),
        idx_sb=idx_sb, sw_sb=sw_sb, cnts=cnts, n_wins=len(wins),
        feat_lo=feat_lo, feat_hi=feat_hi,
        feat_self=feat_self,
    )


def _build_program(N, T, K_LO, K_HI, npc, nw, n_lo, n_hi, n_cores=N_CORES,
                   reps=1, ablate=frozenset()):
    GMAX = 8
    wins = []
    for k0, kn in ((0, K_LO), (K_LO, K_HI)):
        for cb in range(0, kn, GMAX):
            wins.append((k0 + cb, min(GMAX, kn - cb)))
    NWIN = len(wins)
    K = K_LO + K_HI
    K8 = K * 8
    nc = bacc.Bacc(
        "TRN2",
        target_bir_lowering=False,
        debug=False,
        enable_asserts=False,
        num_devices=n_cores,
        num_swdge_queues=4,
        dynamic_dma_scratch_size=32768,
    )

    flo_d = nc.dram_tensor("feat_lo", [n_lo, 256], BF16, kind="ExternalInput")
    fhi_d = nc.dram_tensor("feat_hi", [n_hi, 256], BF16, kind="ExternalInput")
    idx_d = nc.dram_tensor("idx_sb", [P, T * K8], I16, kind="ExternalInput")
    sw_d = nc.dram_tensor("sw_sb", [T * P, K * P], BF16, kind="ExternalInput")
    fself_d = nc.dram_tensor("feat_self", [P, nw], BF16, kind="ExternalInput")
    cnt_d = nc.dram_tensor("cnts", [1, T * NWIN], mybir.dt.int32,
                           kind="ExternalInput")
    wn_d = nc.dram_tensor("wn_t", [P, P], BF16, kind="ExternalInput")
    ws_d = nc.dram_tensor("ws_t", [P, P], BF16, kind="ExternalInput")
    bias_d = nc.dram_tensor("bias_sum", [P, 1], F32, kind="ExternalInput")
    gamma_d = nc.dram_tensor("gamma_c", [P, 1], F32, kind="ExternalInput")
    beta_d = nc.dram_tensor("beta_c", [P, 1], F32, kind="ExternalInput")

    nchunks_out = (npc + LIN_CHUNK - 1) // LIN_CHUNK
    out_d = nc.dram_tensor("outT", [nchunks_out * P, LIN_CHUNK], BF16,
                           kind="ExternalOutput")

    cc_in = nc.dram_tensor("cc_in", [P, 2], F32)
    cc_out = nc.dram_tensor("cc_out", [P, 2], F32, addr_space="Shared")

    with tile.TileContext(nc) as tc, ExitStack() as ctx:
        const = ctx.enter_context(tc.tile_pool(name="const", bufs=1))
        slabs = ctx.enter_context(tc.tile_pool(name="slabs", bufs=1))
        gpool = ctx.enter_context(tc.tile_pool(name="gpool", bufs=4))
        spool = ctx.enter_context(tc.tile_pool(name="spool", bufs=4))
        hnpool = ctx.enter_context(tc.tile_pool(name="hnpool", bufs=3))
        small = ctx.enter_context(tc.tile_pool(name="small", bufs=6))
        stage = ctx.enter_context(tc.tile_pool(name="stage", bufs=3))
        ps_acc = ctx.enter_context(tc.tile_pool(name="ps_acc", bufs=2, space="PSUM"))
        ps_tr = ctx.enter_context(tc.tile_pool(name="ps_tr", bufs=2, space="PSUM"))
        ps_lin = ctx.enter_context(tc.tile_pool(name="ps_lin", bufs=2, space="PSUM"))

        # ---- constants ----
        idx_t = const.tile([P, T * K8], I16)
        IQ = (T + 3) // 4 * K8
        for j in range(4):
            j0 = j * IQ
            j1 = min(T * K8, j0 + IQ)
            if j1 > j0:
                nc.sync.dma_start(idx_t[:, j0:j1], idx_d[:, j0:j1])
        wn_t = const.tile([P, P], BF16)
        nc.sync.dma_start(wn_t[:], wn_d[:, :])
        ws_t = const.tile([P, P], BF16)
        nc.sync.dma_start(ws_t[:], ws_d[:, :])
        bias_t = const.tile([P, 1], F32)
        nc.sync.dma_start(bias_t[:], bias_d[:, :])
        gamma_t = const.tile([P, 1], F32)
        nc.sync.dma_start(gamma_t[:], gamma_d[:, :])
        beta_t = const.tile([P, 1], F32)
        nc.sync.dma_start(beta_t[:], beta_d[:, :])
        ident = const.tile([P, P], BF16)
        make_identity(nc, ident[:])
        cnt_t = const.tile([1, T * NWIN], mybir.dt.int32)
        nc.sync.dma_start(cnt_t[:], cnt_d[:, :])
        nregs = [nc.alloc_register(mybir.EngineType.Pool, f"nidx{i}")
                 for i in range(8)]

        featT = slabs.tile([P, nw], BF16)
        nc.sync.dma_start(featT[:], fself_d[:, :])
        rst = slabs.tile([P, nw], F32)
        hnT = slabs.tile([P, nw], BF16)

        # ablation shrink factors (bench-only; full kernel uses none)
        AB_G = "gather" in ablate
        AB_MM = "mm" in ablate
        AB_EPI = "epi" in ablate

        gq = [0]  # round-robin SWDGE queue so desc-gen pipelines across Q7 core pairs

        # zero the gather buffers once: slots whose descriptors are skipped
        # (trailing -1 idxs) keep stale SBUF, and 0 * NaN would poison PSUM
        for _b in range(4):
            gz = gpool.tile([P, K, 256], BF16)
            nc.vector.memset(gz[:], 0.0)

        nchunks = (nw + LIN_CHUNK - 1) // LIN_CHUNK

        def lin_chunk(j, sum_parts, sq_parts):
            """fc_self + fc_neigh + bias + relu + BN partial stats for column
            chunk j.  Interleaved into the tile loop so the epilogue overlaps
            message passing instead of trailing it."""
            c0 = j * LIN_CHUNK
            cw = min(LIN_CHUNK, nw - c0)
            vw = min(max(npc - c0, 0), cw)          # valid (non-pad) columns
            pl = ps_lin.tile([P, LIN_CHUNK], F32, space="PSUM")
            nc.tensor.matmul(
                out=pl[:, 0:cw], lhsT=ws_t[:], rhs=featT[:, c0:c0 + cw],
                start=True, stop=False,
            )
            nc.tensor.matmul(
                out=pl[:, 0:cw], lhsT=wn_t[:], rhs=hnT[:, c0:c0 + cw],
                start=False, stop=True,
            )
            # rst = relu(pl + bias); partial sums on ACT
            nc.scalar.activation(
                out=rst[:, c0:c0 + cw], in_=pl[:, 0:cw], func=ACT.Relu,
                bias=bias_t[:],
            )
            if vw > 0:
                nc.vector.tensor_reduce(
                    out=sum_parts[:, j:j + 1], in_=rst[:, c0:c0 + vw],
                    axis=mybir.AxisListType.X, op=OP.add,
                )
                junk = stage.tile([P, LIN_CHUNK], F32, tag="junk")
                nc.scalar.activation(
                    out=junk[:, 0:vw], in_=rst[:, c0:c0 + vw], func=ACT.Square,
                    accum_out=sq_parts[:, j:j + 1],
                )
            else:
                nc.vector.memset(sum_parts[:, j:j + 1], 0.0)
                nc.vector.memset(sq_parts[:, j:j + 1], 0.0)

        for _rep in range(reps):
            sum_parts = small.tile([P, nchunks], F32, tag="sump")
            sq_parts = small.tile([P, nchunks], F32, tag="sqp")
            emitted = [0]
            # ---- message passing per dst tile ----
            for t in range(T):
                g = gpool.tile([P, K, 256], BF16)
                for wi, (cb, cn) in enumerate(wins):
                    tab = flo_d if cb < K_LO else fhi_d
                    r = nregs[gq[0] % 8]
                    nc.gpsimd.reg_load(
                        r, cnt_t[0:1, t * NWIN + wi:t * NWIN + wi + 1])
                    nc.gpsimd.dma_gather(
                        out_ap=g[:, cb:cb + cn, :],
                        in_ap=tab.ap(),
                        idxs_ap=idx_t[:, t * K8 + cb * 8:
                                      t * K8 + (cb + cn) * 8],
                        num_idxs=cn * P,
                        num_idxs_reg=r,
                        elem_size=256,
                        queue_num=gq[0] % 4,
                    )
                    gq[0] += 1
                # host-built weighted one-hot for this tile
                s = spool.tile([P, K * P], BF16)
                nc.sync.dma_start(s[:], sw_d[t * P:(t + 1) * P, :])
                # accumulate [dst, feat | deg]: rhs column 128 is the 1.0 pad
                ps = ps_acc.tile([P, P + 1], F32, space="PSUM")
                K_MM = 1 if AB_MM else K
                for c in range(K_MM):
                    nc.tensor.matmul(
                        out=ps[:],
                        lhsT=s[:, c * P:(c + 1) * P],
                        rhs=g[:, c, 0:P + 1],
                        start=(c == 0),
                        stop=(c == K_MM - 1),
                    )
                dinv = small.tile([P, 1], F32, tag="dinv")
                nc.vector.tensor_scalar(
                    out=dinv[:], in0=ps[:, P:P + 1], scalar1=EPS_DEG, scalar2=None,
                    op0=OP.add,
                )
                nc.vector.reciprocal(dinv[:], dinv[:])
                hn = hnpool.tile([P, P], BF16)
                nc.scalar.activation(
                    out=hn[:], in_=ps[:, 0:P], func=ACT.Copy, scale=dinv[:],
                )
                pst = ps_tr.tile([P, P], BF16, space="PSUM")
                nc.tensor.transpose(out=pst[:], in_=hn[:], identity=ident[:])
                nc.scalar.copy(hnT[:, t * P:(t + 1) * P], pst[:])

                # epilogue chunk j is ready once its 4 source tiles are done
                while (emitted[0] + 1) * (LIN_CHUNK // P) <= t + 1 \
                        and emitted[0] < nchunks:
                    lin_chunk(emitted[0], sum_parts, sq_parts)
                    emitted[0] += 1

            while emitted[0] < nchunks:
                lin_chunk(emitted[0], sum_parts, sq_parts)
                emitted[0] += 1

            stats = small.tile([P, 2], F32, tag="stats")
            nc.vector.tensor_reduce(
                out=stats[:, 0:1], in_=sum_parts[:, 0:nchunks],
                axis=mybir.AxisListType.X, op=OP.add
            )
            nc.vector.tensor_reduce(
                out=stats[:, 1:2], in_=sq_parts[:, 0:nchunks],
                axis=mybir.AxisListType.X, op=OP.add
            )
            nc.sync.dma_start(cc_in[:, :], stats[:])
            nc.gpsimd.collective_compute(
                "AllReduce",
                OP.add,
                replica_groups=[list(range(n_cores))],
                ins=[cc_in.ap().opt()],
                outs=[cc_out.ap().opt()],
            )
            gstats = small.tile([P, 2], F32, tag="gstats")
            nc.sync.dma_start(gstats[:], cc_out[:, :])

            # ---- BN scale/shift ----
            inv_n = 1.0 / N
            mu = small.tile([P, 1], F32, tag="mu")
            nc.vector.tensor_scalar(
                out=mu[:], in0=gstats[:, 0:1], scalar1=inv_n, scalar2=None, op0=OP.mult
            )
            var = small.tile([P, 1], F32, tag="var")
            nc.vector.tensor_scalar(
                out=var[:], in0=gstats[:, 1:2], scalar1=inv_n, scalar2=None, op0=OP.mult
            )
            mu2 = small.tile([P, 1], F32, tag="mu2")
            nc.vector.tensor_tensor(out=mu2[:], in0=mu[:], in1=mu[:], op=OP.mult)
            nc.vector.tensor_tensor(out=var[:], in0=var[:], in1=mu2[:], op=OP.subtract)
            eps_t = small.tile([P, 1], F32, tag="eps")
            nc.vector.memset(eps_t[:], EPS_BN)
            std = small.tile([P, 1], F32, tag="std")
            nc.scalar.activation(out=std[:], in_=var[:], func=ACT.Sqrt, bias=eps_t[:])
            rstd = small.tile([P, 1], F32, tag="rstd")
            nc.vector.reciprocal(rstd[:], std[:])
            scale = small.tile([P, 1], F32, tag="scale")
            nc.vector.tensor_tensor(out=scale[:], in0=gamma_t[:], in1=rstd[:], op=OP.mult)
            shift = small.tile([P, 1], F32, tag="shift")
            nc.vector.tensor_tensor(out=shift[:], in0=mu[:], in1=scale[:], op=OP.mult)
            nc.vector.tensor_tensor(out=shift[:], in0=beta_t[:], in1=shift[:], op=OP.subtract)

            # ---- apply + write out (contiguous chunk layout) ----
            for j in range(1 if AB_EPI else (npc + LIN_CHUNK - 1) // LIN_CHUNK):
                c0 = j * LIN_CHUNK
                cw = min(LIN_CHUNK, npc - c0)
                ot = stage.tile([P, LIN_CHUNK], BF16, tag="ostage")
                nc.vector.tensor_scalar(
                    out=ot[:, 0:cw], in0=rst[:, c0:c0 + cw],
                    scalar1=scale[:], scalar2=shift[:], op0=OP.mult, op1=OP.add,
                )
                nc.sync.dma_start(out_d[j * P:(j + 1) * P, 0:cw], ot[:, 0:cw])

    nc.compile()
    return nc


_cache = {}


def _get_program(key_params):
    key = tuple(sorted(key_params.items()))
    if key not in _cache:
        _cache[key] = _build_program(**key_params)
    return _cache[key]


def _in_maps(plan, W_neigh, W_self, b_self, bias, gamma, beta):
    wn_t = np.ascontiguousarray(W_neigh.T).astype(ml_dtypes.bfloat16)
    ws_t = np.ascontiguousarray(W_self.T).astype(ml_dtypes.bfloat16)
    bias_sum = (np.asarray(b_self) + np.asarray(bias)).astype(np.float32).reshape(P, 1)
    maps = []
    for c in range(N_CORES):
        maps.append({
            "feat_lo": plan["feat_lo"],
            "feat_hi": plan["feat_hi"],
            "idx_sb": plan["idx_sb"][c],
            "sw_sb": plan["sw_sb"][c],
            "cnts": plan["cnts"][c],
            "feat_self": plan["feat_self"][c],
            "wn_t": wn_t,
            "ws_t": ws_t,
            "bias_sum": bias_sum,
            "gamma_c": np.asarray(gamma, np.float32).reshape(P, 1),
            "beta_c": np.asarray(beta, np.float32).reshape(P, 1),
        })
    return maps


def kernel(feat, src, dst, edge_weight, W_neigh, W_self, b_self, bias, gamma, beta):
    N, D = feat.shape
    plan = _host_plan(
        np.asarray(feat), np.asarray(src), np.asarray(dst), np.asarray(edge_weight)
    )
    npc = plan["npc"]

    nc = _get_program(dict(
        N=N, T=plan["T"], K_LO=plan["K_LO"], K_HI=plan["K_HI"],
        npc=npc, nw=plan["nw"], n_lo=plan["n_lo"], n_hi=plan["n_hi"],
    ))

    maps = _in_maps(plan, W_neigh, W_self, b_self, bias, gamma, beta)
    res = run_bass_kernel_spmd(nc, maps, core_ids=list(range(N_CORES)))
    out = np.empty((N, P), np.float32)
    ncho = (npc + LIN_CHUNK - 1) // LIN_CHUNK
    for c in range(N_CORES):
        chunks = res.results[c]["outT"].astype(np.float32).reshape(
            ncho, P, LIN_CHUNK)
        outT = np.concatenate(
            [chunks[j][:, :min(LIN_CHUNK, npc - j * LIN_CHUNK)]
             for j in range(ncho)], axis=1,
        )
        out[c * npc:(c + 1) * npc] = outT.T
    return out



# revision 20
# speedup vs baseline: 1.7484x; 1.0128x over previous
"""Trainium2 Bass kernel for GNN message passing (IntraConv + BatchNorm).

Computation (reference):
    msg   = feat[src] * edge_weight                    [E, D]
    neigh = segment_sum(msg, dst, N)                   [N, D]
    deg   = segment_sum(edge_weight, dst, N)           [N, 1]
    h     = relu(feat @ Ws.T + b_self + (neigh/(deg+eps)) @ Wn.T + bias)
    out   = batchnorm(h; gamma, beta)  (training-mode batch stats)

Distribution over 8 NeuronCores: edges are sorted by dst and sharded by
dst-range so each core owns N/8 contiguous nodes and every edge pointing at
them.  Local segment sums are then exact — the only collective is an
AllReduce of the [128, 2] BatchNorm statistics.

Per-core pipeline (feature-major):
  - dma_gather of fp32 feature rows (512B) per 128-dst tile.  dma_gather
    indices are int16, so the node table is split at 32768 and each tile
    does a lo-gather and a hi-gather into one [128, K, 128] buffer.
  - one-hot S[e, d] = (dstl[e] == d) built in bf16 with a single is_equal;
    PE matmuls accumulate S.T @ (w * G) into PSUM [128 nodes, 128] (neigh)
    and S.T @ w into PSUM [128, 1] (degree).
  - normalize by 1/(deg+eps) (per-partition scalar), transpose to
    feature-major via PE, assemble h_neigh.T slab.
  - linears with stationary W.T (bf16); bias+relu and BN partial stats on
    the ACT engine (activation Relu/Square with accum_out); tiny AllReduce;
    scale/shift; output written feature-major [128, N/8] and transposed on
    the host during unshard.
"""

import numpy as np
import ml_dtypes
from contextlib import ExitStack

import concourse.bass as bass
import concourse.tile as tile
from concourse import bacc, mybir
from concourse.bass_utils import run_bass_kernel_spmd
from concourse.masks import make_identity

N_CORES = 8
P = 128
HALF = 32768        # int16 index limit for dma_gather
LIN_CHUNK = 512
EPS_DEG = 1e-8
EPS_BN = 1e-5

F32 = mybir.dt.float32
BF16 = mybir.dt.bfloat16
I16 = mybir.dt.int16
OP = mybir.AluOpType
ACT = mybir.ActivationFunctionType


def _bcast_inner(ap, n):
    """[.., M] -> [.., M, n] with stride-0 inner broadcast dim."""
    return bass.AP(tensor=ap.tensor, offset=ap.offset, ap=list(ap.ap) + [[0, n]])


def _bcast_mid(ap2d, k):
    """[Pp, M] -> [Pp, k(bcast), M]."""
    a = list(ap2d.ap)
    return bass.AP(tensor=ap2d.tensor, offset=ap2d.offset, ap=[a[0], [0, k], a[1]])


def _host_plan(feat, src, dst, edge_weight):
    N, D = feat.shape
    E = src.shape[0]
    assert D == P and N % N_CORES == 0
    npc = N // N_CORES                      # nodes per core
    T = (npc + P - 1) // P                  # dst tiles per core
    nw = T * P                              # padded node-slab width
    n_hi = N - HALF if N > HALF else 0

    src64 = src.astype(np.int64)
    dst64 = dst.astype(np.int64)
    ws_all = edge_weight.reshape(-1).astype(np.float32)

    half = (src64 >= HALF).astype(np.int64)
    ct = (dst64 // npc) * T + (dst64 % npc) // P      # (core, tile) group id
    order = np.lexsort((half, ct))
    ss = src64[order]
    ws = ws_all[order]
    hh = half[order]
    cts = ct[order]
    dstl = ((dst64[order] % npc) % P).astype(np.float32)

    grp = cts * 2 + hh                                 # (core, tile, half)
    counts = np.bincount(grp, minlength=N_CORES * T * 2)
    cnt_lo = counts[0::2]
    cnt_hi = counts[1::2]
    K_LO = max(1, int(np.ceil(cnt_lo.max() / P)))
    K_HI = max(1, int(np.ceil(cnt_hi.max() / P))) if n_hi > 0 else 0
    K = K_LO + K_HI
    ET = K * P

    starts = np.zeros(N_CORES * T * 2 + 1, np.int64)
    np.cumsum(counts, out=starts[1:])
    pos = np.arange(E, dtype=np.int64) - starts[grp]
    q = pos + hh * (K_LO * P)                          # stream position in tile
    flat = cts * ET + q

    # pad slots: idx=-1.  Trailing negatives are skipped by the Q7 desc-gen;
    # num_idxs_reg is reg_loaded per core with the exact valid count so the NX
    # ring bookkeeping matches the Q7's trailing-negative scan.
    idx_stream = np.full(N_CORES * T * ET, -1, np.int32)
    idx_stream[flat] = ss - hh * HALF

    # per-(core, tile, gather-window) valid counts, in gather emission order
    GMAX = 8
    wins = []
    for k0, kn in ((0, K_LO), (K_LO, K_HI)):
        for cb in range(0, kn, GMAX):
            wins.append((k0 + cb, min(GMAX, kn - cb)))
    cl = cnt_lo.reshape(N_CORES, T)
    ch = cnt_hi.reshape(N_CORES, T)
    cnts = np.zeros((N_CORES, T, len(wins)), np.int32)
    for wi, (cb, cn) in enumerate(wins):
        if cb < K_LO:
            base, seg0 = cl, cb
        else:
            base, seg0 = ch, cb - K_LO
        cnts[:, :, wi] = np.clip(base - seg0 * P, 0, cn * P)
    cnts = cnts.reshape(N_CORES, 1, T * len(wins))

    # host-built weighted one-hot: S_w[core, p, t*K + c, dstl] = w for the edge
    # at stream position q = c*128 + p of (core, tile).  Pure placement of w
    # values (no arithmetic); replaces the per-tile DVE is_equal/mult build.
    core_of = cts // T
    t_of = cts % T
    c_of = q // P
    p_of = q % P
    sw_sb = np.zeros((N_CORES, T, P, K, P), ml_dtypes.bfloat16)
    sw_sb[core_of, t_of, p_of, c_of, dstl.astype(np.int64)] = ws.astype(
        ml_dtypes.bfloat16
    )
    # tile-major contiguous: each tile's [128, K*128] block is one flat DRAM
    # run so its load coalesces into 16 large descriptors, not a 128-way spray
    sw_sb = np.ascontiguousarray(sw_sb.reshape(N_CORES, T * P, K * P))

    # gather indices, packed by SWDGE queue: window w rides queue w%4, whose
    # Q7 pair (cores 2q, 2q+1) reads only partitions [32q, 32q+32).  Each
    # window's 16-wrap is placed twice (tx + rx core slices) in its queue's
    # partition band; windows of different queues share column blocks.
    ist = idx_stream.reshape(N_CORES, T, ET)
    NB = (T * len(wins) + 3) // 4
    WCOL = GMAX * 8
    idx_sb = np.zeros((N_CORES, P, NB * WCOL), np.int16)
    w = 0
    for t in range(T):
        for cb, cn in wins:
            qn, b = w % 4, w // 4
            seg = ist[:, t, cb * P:(cb + cn) * P].reshape(N_CORES, cn * 8, 16)
            seg = seg.transpose(0, 2, 1).astype(np.int16)
            for rep in range(2):
                p0 = 32 * qn + 16 * rep
                idx_sb[:, p0:p0 + 16, b * WCOL:b * WCOL + cn * 8] = seg
            w += 1

    # bf16 tables with 256-elem rows: [feat(128), 1.0, zeros(127)].
    # 512B/row keeps the gather at full descriptor rate; the 1.0 column makes
    # the degree fall out of the neighbor matmul (rhs column 128).
    def table(rows):
        t = np.zeros((max(rows.shape[0], 1), 256), ml_dtypes.bfloat16)
        t[: rows.shape[0], :P] = rows.astype(ml_dtypes.bfloat16)
        t[:, P] = 1.0
        return t

    feat_lo = table(feat[:HALF])
    feat_hi = table(feat[HALF:]) if n_hi > 0 else np.zeros((1, 256), ml_dtypes.bfloat16)

    # per-core self-feature slab, bf16, pre-transposed to [P, nw] so the load
    # is a plain contiguous DMA instead of an xbar transpose
    feat_self = np.zeros((N_CORES, P, nw), ml_dtypes.bfloat16)
    fb = feat.reshape(N_CORES, npc, P)
    for c in range(N_CORES):
        feat_self[c, :, :npc] = fb[c].T

    return dict(
        N=N, E=E, npc=npc, T=T, K_LO=K_LO, K_HI=K_HI, nw=nw,
        n_lo=min(N, HALF), n_hi=max(n_hi, 1),
        idx_sb=idx_sb, sw_sb=sw_sb, cnts=cnts, n_wins=len(wins),
        feat_lo=feat_lo, feat_hi=feat_hi,
        feat_self=feat_self,
    )


def _build_program(N, T, K_LO, K_HI, npc, nw, n_lo, n_hi, n_cores=N_CORES,
                   reps=1, ablate=frozenset()):
    GMAX = 8
    wins = []
    for k0, kn in ((0, K_LO), (K_LO, K_HI)):
        for cb in range(0, kn, GMAX):
            wins.append((k0 + cb, min(GMAX, kn - cb)))
    NWIN = len(wins)
    K = K_LO + K_HI
    K8 = K * 8
    nc = bacc.Bacc(
        "TRN2",
        target_bir_lowering=False,
        debug=False,
        enable_asserts=False,
        num_devices=n_cores,
        num_swdge_queues=4,
        dynamic_dma_scratch_size=32768,
    )

    flo_d = nc.dram_tensor("feat_lo", [n_lo, 256], BF16, kind="ExternalInput")
    fhi_d = nc.dram_tensor("feat_hi", [n_hi, 256], BF16, kind="ExternalInput")
    NB = (T * NWIN + 3) // 4
    WCOL = GMAX * 8
    idx_d = nc.dram_tensor("idx_sb", [P, NB * WCOL], I16, kind="ExternalInput")
    sw_d = nc.dram_tensor("sw_sb", [T * P, K * P], BF16, kind="ExternalInput")
    fself_d = nc.dram_tensor("feat_self", [P, nw], BF16, kind="ExternalInput")
    cnt_d = nc.dram_tensor("cnts", [1, T * NWIN], mybir.dt.int32,
                           kind="ExternalInput")
    wn_d = nc.dram_tensor("wn_t", [P, P], BF16, kind="ExternalInput")
    ws_d = nc.dram_tensor("ws_t", [P, P], BF16, kind="ExternalInput")
    bias_d = nc.dram_tensor("bias_sum", [P, 1], F32, kind="ExternalInput")
    gamma_d = nc.dram_tensor("gamma_c", [P, 1], F32, kind="ExternalInput")
    beta_d = nc.dram_tensor("beta_c", [P, 1], F32, kind="ExternalInput")

    nchunks_out = (npc + LIN_CHUNK - 1) // LIN_CHUNK
    out_d = nc.dram_tensor("outT", [nchunks_out * P, LIN_CHUNK], BF16,
                           kind="ExternalOutput")

    cc_in = nc.dram_tensor("cc_in", [P, 2], F32)
    cc_out = nc.dram_tensor("cc_out", [P, 2], F32, addr_space="Shared")

    with tile.TileContext(nc) as tc, ExitStack() as ctx:
        const = ctx.enter_context(tc.tile_pool(name="const", bufs=1))
        slabs = ctx.enter_context(tc.tile_pool(name="slabs", bufs=1))
        gpool = ctx.enter_context(tc.tile_pool(name="gpool", bufs=5))
        spool = ctx.enter_context(tc.tile_pool(name="spool", bufs=4))
        hnpool = ctx.enter_context(tc.tile_pool(name="hnpool", bufs=3))
        small = ctx.enter_context(tc.tile_pool(name="small", bufs=6))
        stage = ctx.enter_context(tc.tile_pool(name="stage", bufs=3))
        ps_acc = ctx.enter_context(tc.tile_pool(name="ps_acc", bufs=2, space="PSUM"))
        ps_tr = ctx.enter_context(tc.tile_pool(name="ps_tr", bufs=2, space="PSUM"))
        ps_lin = ctx.enter_context(tc.tile_pool(name="ps_lin", bufs=2, space="PSUM"))

        # ---- constants ----
        idx_t = const.tile([P, NB * WCOL], I16)
        IQ = (NB * WCOL + 3) // 4
        for j in range(4):
            j0 = j * IQ
            j1 = min(NB * WCOL, j0 + IQ)
            if j1 > j0:
                nc.sync.dma_start(idx_t[:, j0:j1], idx_d[:, j0:j1])
        wn_t = const.tile([P, P], BF16)
        nc.sync.dma_start(wn_t[:], wn_d[:, :])
        ws_t = const.tile([P, P], BF16)
        nc.sync.dma_start(ws_t[:], ws_d[:, :])
        bias_t = const.tile([P, 1], F32)
        nc.sync.dma_start(bias_t[:], bias_d[:, :])
        gamma_t = const.tile([P, 1], F32)
        nc.sync.dma_start(gamma_t[:], gamma_d[:, :])
        beta_t = const.tile([P, 1], F32)
        nc.sync.dma_start(beta_t[:], beta_d[:, :])
        ident = const.tile([P, P], BF16)
        make_identity(nc, ident[:])
        cnt_t = const.tile([1, T * NWIN], mybir.dt.int32)
        nc.sync.dma_start(cnt_t[:], cnt_d[:, :])
        nregs = [nc.alloc_register(mybir.EngineType.Pool, f"nidx{i}")
                 for i in range(8)]

        featT = slabs.tile([P, nw], BF16)
        nc.sync.dma_start(featT[:], fself_d[:, :])
        rst = slabs.tile([P, nw], F32)
        hnT = slabs.tile([P, nw], BF16)

        # ablation shrink factors (bench-only; full kernel uses none)
        AB_G = "gather" in ablate
        AB_MM = "mm" in ablate
        AB_EPI = "epi" in ablate

        gq = [0]  # round-robin SWDGE queue so desc-gen pipelines across Q7 core pairs

        # zero the gather buffers once: slots whose descriptors are skipped
        # (trailing -1 idxs) keep stale SBUF, and 0 * NaN would poison PSUM
        for _b in range(5):
            gz = gpool.tile([P, K, 256], BF16)
            nc.scalar.memzero(gz[:])

        nchunks = (nw + LIN_CHUNK - 1) // LIN_CHUNK

        def lin_chunk(j, sum_parts, sq_parts):
            """fc_self + fc_neigh + bias + relu + BN partial stats for column
            chunk j.  Interleaved into the tile loop so the epilogue overlaps
            message passing instead of trailing it."""
            c0 = j * LIN_CHUNK
            cw = min(LIN_CHUNK, nw - c0)
            vw = min(max(npc - c0, 0), cw)          # valid (non-pad) columns
            pl = ps_lin.tile([P, LIN_CHUNK], F32, space="PSUM")
            nc.tensor.matmul(
                out=pl[:, 0:cw], lhsT=ws_t[:], rhs=featT[:, c0:c0 + cw],
                start=True, stop=False,
            )
            nc.tensor.matmul(
                out=pl[:, 0:cw], lhsT=wn_t[:], rhs=hnT[:, c0:c0 + cw],
                start=False, stop=True,
            )
            # rst = relu(pl + bias); partial sums on ACT
            nc.scalar.activation(
                out=rst[:, c0:c0 + cw], in_=pl[:, 0:cw], func=ACT.Relu,
                bias=bias_t[:],
            )
            if vw > 0:
                nc.vector.tensor_reduce(
                    out=sum_parts[:, j:j + 1], in_=rst[:, c0:c0 + vw],
                    axis=mybir.AxisListType.X, op=OP.add,
                )
                junk = stage.tile([P, LIN_CHUNK], F32, tag="junk")
                nc.scalar.activation(
                    out=junk[:, 0:vw], in_=rst[:, c0:c0 + vw], func=ACT.Square,
                    accum_out=sq_parts[:, j:j + 1],
                )
            else:
                nc.vector.memset(sum_parts[:, j:j + 1], 0.0)
                nc.vector.memset(sq_parts[:, j:j + 1], 0.0)

        for _rep in range(reps):
            sum_parts = small.tile([P, nchunks], F32, tag="sump")
            sq_parts = small.tile([P, nchunks], F32, tag="sqp")
            emitted = [0]
            # ---- message passing per dst tile ----
            for t in range(T):
                g = gpool.tile([P, K, 256], BF16)
                for wi, (cb, cn) in enumerate(wins):
                    tab = flo_d if cb < K_LO else fhi_d
                    r = nregs[gq[0] % 8]
                    b = gq[0] // 4
                    nc.gpsimd.reg_load(
                        r, cnt_t[0:1, t * NWIN + wi:t * NWIN + wi + 1])
                    nc.gpsimd.dma_gather(
                        out_ap=g[:, cb:cb + cn, :],
                        in_ap=tab.ap(),
                        idxs_ap=idx_t[:, b * WCOL:b * WCOL + cn * 8],
                        num_idxs=cn * P,
                        num_idxs_reg=r,
                        elem_size=256,
                        queue_num=gq[0] % 4,
                    )
                    gq[0] += 1
                # host-built weighted one-hot for this tile
                s = spool.tile([P, K * P], BF16)
                nc.sync.dma_start(s[:], sw_d[t * P:(t + 1) * P, :])
                # accumulate [dst, feat | deg]: rhs column 128 is the 1.0 pad
                ps = ps_acc.tile([P, P + 1], F32, space="PSUM")
                K_MM = 1 if AB_MM else K
                for c in range(K_MM):
                    nc.tensor.matmul(
                        out=ps[:],
                        lhsT=s[:, c * P:(c + 1) * P],
                        rhs=g[:, c, 0:P + 1],
                        start=(c == 0),
                        stop=(c == K_MM - 1),
                    )
                dinv = small.tile([P, 1], F32, tag="dinv")
                nc.vector.tensor_scalar(
                    out=dinv[:], in0=ps[:, P:P + 1], scalar1=EPS_DEG, scalar2=None,
                    op0=OP.add,
                )
                nc.vector.reciprocal(dinv[:], dinv[:])
                hn = hnpool.tile([P, P], BF16)
                nc.scalar.activation(
                    out=hn[:], in_=ps[:, 0:P], func=ACT.Copy, scale=dinv[:],
                )
                pst = ps_tr.tile([P, P], BF16, space="PSUM")
                nc.tensor.transpose(out=pst[:], in_=hn[:], identity=ident[:])
                nc.scalar.copy(hnT[:, t * P:(t + 1) * P], pst[:])

                # epilogue chunk j is ready once its 4 source tiles are done
                while (emitted[0] + 1) * (LIN_CHUNK // P) <= t + 1 \
                        and emitted[0] < nchunks:
                    lin_chunk(emitted[0], sum_parts, sq_parts)
                    emitted[0] += 1

            while emitted[0] < nchunks:
                lin_chunk(emitted[0], sum_parts, sq_parts)
                emitted[0] += 1

            stats = small.tile([P, 2], F32, tag="stats")
            nc.vector.tensor_reduce(
                out=stats[:, 0:1], in_=sum_parts[:, 0:nchunks],
                axis=mybir.AxisListType.X, op=OP.add
            )
            nc.vector.tensor_reduce(
                out=stats[:, 1:2], in_=sq_parts[:, 0:nchunks],
                axis=mybir.AxisListType.X, op=OP.add
            )
            nc.sync.dma_start(cc_in[:, :], stats[:])
            nc.gpsimd.collective_compute(
                "AllReduce",
                OP.add,
                replica_groups=[list(range(n_cores))],
                ins=[cc_in.ap().opt()],
                outs=[cc_out.ap().opt()],
            )
            gstats = small.tile([P, 2], F32, tag="gstats")
            nc.sync.dma_start(gstats[:], cc_out[:, :])

            # ---- BN scale/shift ----
            inv_n = 1.0 / N
            mu = small.tile([P, 1], F32, tag="mu")
            nc.vector.tensor_scalar(
                out=mu[:], in0=gstats[:, 0:1], scalar1=inv_n, scalar2=None, op0=OP.mult
            )
            var = small.tile([P, 1], F32, tag="var")
            nc.vector.tensor_scalar(
                out=var[:], in0=gstats[:, 1:2], scalar1=inv_n, scalar2=None, op0=OP.mult
            )
            mu2 = small.tile([P, 1], F32, tag="mu2")
            nc.vector.tensor_tensor(out=mu2[:], in0=mu[:], in1=mu[:], op=OP.mult)
            nc.vector.tensor_tensor(out=var[:], in0=var[:], in1=mu2[:], op=OP.subtract)
            eps_t = small.tile([P, 1], F32, tag="eps")
            nc.vector.memset(eps_t[:], EPS_BN)
            std = small.tile([P, 1], F32, tag="std")
            nc.scalar.activation(out=std[:], in_=var[:], func=ACT.Sqrt, bias=eps_t[:])
            rstd = small.tile([P, 1], F32, tag="rstd")
            nc.vector.reciprocal(rstd[:], std[:])
            scale = small.tile([P, 1], F32, tag="scale")
            nc.vector.tensor_tensor(out=scale[:], in0=gamma_t[:], in1=rstd[:], op=OP.mult)
            shift = small.tile([P, 1], F32, tag="shift")
            nc.vector.tensor_tensor(out=shift[:], in0=mu[:], in1=scale[:], op=OP.mult)
            nc.vector.tensor_tensor(out=shift[:], in0=beta_t[:], in1=shift[:], op=OP.subtract)

            # ---- apply + write out (contiguous chunk layout) ----
            # alternate DVE / ACT so the serial tail halves
            for j in range(1 if AB_EPI else (npc + LIN_CHUNK - 1) // LIN_CHUNK):
                c0 = j * LIN_CHUNK
                cw = min(LIN_CHUNK, npc - c0)
                ot = stage.tile([P, LIN_CHUNK], BF16, tag="ostage")
                if j % 2 == 0:
                    nc.vector.tensor_scalar(
                        out=ot[:, 0:cw], in0=rst[:, c0:c0 + cw],
                        scalar1=scale[:], scalar2=shift[:],
                        op0=OP.mult, op1=OP.add,
                    )
                else:
                    nc.scalar.activation(
                        out=ot[:, 0:cw], in_=rst[:, c0:c0 + cw],
                        func=ACT.Identity, scale=scale[:], bias=shift[:],
                    )
                nc.sync.dma_start(out_d[j * P:(j + 1) * P, 0:cw], ot[:, 0:cw])

    nc.compile()
    return nc


_cache = {}


def _get_program(key_params):
    key = tuple(sorted(key_params.items()))
    if key not in _cache:
        _cache[key] = _build_program(**key_params)
    return _cache[key]


def _in_maps(plan, W_neigh, W_self, b_self, bias, gamma, beta):
    wn_t = np.ascontiguousarray(W_neigh.T).astype(ml_dtypes.bfloat16)
    ws_t = np.ascontiguousarray(W_self.T).astype(ml_dtypes.bfloat16)
    bias_sum = (np.asarray(b_self) + np.asarray(bias)).astype(np.float32).reshape(P, 1)
    maps = []
    for c in range(N_CORES):
        maps.append({
            "feat_lo": plan["feat_lo"],
            "feat_hi": plan["feat_hi"],
            "idx_sb": plan["idx_sb"][c],
            "sw_sb": plan["sw_sb"][c],
            "cnts": plan["cnts"][c],
            "feat_self": plan["feat_self"][c],
            "wn_t": wn_t,
            "ws_t": ws_t,
            "bias_sum": bias_sum,
            "gamma_c": np.asarray(gamma, np.float32).reshape(P, 1),
            "beta_c": np.asarray(beta, np.float32).reshape(P, 1),
        })
    return maps


def kernel(feat, src, dst, edge_weight, W_neigh, W_self, b_self, bias, gamma, beta):
    N, D = feat.shape
    plan = _host_plan(
        np.asarray(feat), np.asarray(src), np.asarray(dst), np.asarray(edge_weight)
    )
    npc = plan["npc"]

    nc = _get_program(dict(
        N=N, T=plan["T"], K_LO=plan["K_LO"], K_HI=plan["K_HI"],
        npc=npc, nw=plan["nw"], n_lo=plan["n_lo"], n_hi=plan["n_hi"],
    ))

    maps = _in_maps(plan, W_neigh, W_self, b_self, bias, gamma, beta)
    res = run_bass_kernel_spmd(nc, maps, core_ids=list(range(N_CORES)))
    out = np.empty((N, P), np.float32)
    ncho = (npc + LIN_CHUNK - 1) // LIN_CHUNK
    for c in range(N_CORES):
        chunks = res.results[c]["outT"].astype(np.float32).reshape(
            ncho, P, LIN_CHUNK)
        outT = np.concatenate(
            [chunks[j][:, :min(LIN_CHUNK, npc - j * LIN_CHUNK)]
             for j in range(ncho)], axis=1,
        )
        out[c * npc:(c + 1) * npc] = outT.T
    return out

